# revision 20
# baseline (speedup 1.0000x reference)
"""Trainium2 Bass kernel for 2-layer GATv2 + output projection (SPMD, 8 cores).

v2 strategy: nodes partitioned across cores; per 128-dst-node window, edges
are packed into SW=J*512 slots (sorted by gathered-row id so the low/high
int16 gather split is two window-level dma_gather calls). fp16 data plane
(same mantissa as TF32/f32r), f32 PSUM accumulation. Softmax denominator is
factored out of the weighted scatter (out = (sum ex*xl[src]) / denom), so
each window is a single pass: per 512-slot tile, edge-feature projection +
xr[dst] broadcast (selector matmul) + transposed gathered xl accumulate into
PSUM; fused leaky-relu; per-head logits via PE; exp on ACT; one selector
matmul per 128-slot group scatters both the weighted values and the
denominator (values carry ex in 4 extra columns). Halo exchange = AllGather
of fp16 xl shards. xr and the inter-layer h1 stay SBUF-resident.
"""
import numpy as np

import concourse.bass as bass
import concourse.bacc as bacc
import concourse.mybir as mybir
import concourse.tile as tile
from concourse.bass_utils import run_bass_kernel_spmd
from concourse.masks import make_identity

F32 = mybir.dt.float32
F16 = mybir.dt.float16
I16 = mybir.dt.int16
I32 = mybir.dt.int32

NEG_SLOPE = 0.2
EPS = 1e-30


class Cfg:
    def __init__(self, N, E, IN_F, NC, J, reps=1, NWX=0):
        self.N, self.E, self.IN_F, self.NC, self.J = N, E, IN_F, NC, J
        self.reps = reps
        self.NWX = NWX
        self.F = 256
        self.H, self.C = 4, 64
        self.W = 128                      # dst nodes per window
        assert N % NC == 0
        self.NV = N // NC                 # nodes per core
        self.NW = (self.NV + self.W - 1) // self.W + NWX
        self.NVP = self.NW * self.W       # padded nodes per core
        self.T_E = 512                    # edge slots per tile
        self.G = 4                        # 128-edge groups per tile
        self.SW = self.J * self.T_E       # edge slots per window
        self.SWC = self.SW // 128         # 128-slot chunks per window
        self.NCH = 4                      # gather chunks/window: 2 low, 2 high
        assert self.SW % self.NCH == 0
        self.HSW = self.SW // self.NCH    # slots per gather chunk
        assert self.HSW % 16 == 0 and (self.HSW // 128) * 128 == self.HSW
        self.CAPL = 2 * self.HSW          # low-row slot capacity per window
        self.CAPH = 2 * self.HSW          # high-row slot capacity per window
        self.NFULL = self.NC * self.NVP   # rows in gathered xl table
        assert self.NFULL % 2 == 0
        self.HALF = self.NFULL // 2
        assert self.HALF <= 32768, "int16 gather index range"
        self.KCH = self.IN_F // 128       # K-chunks for layer-0 projection
        import os
        _ph = os.environ.get("K_PHASES", "p1,ag0,e0,p4,ag1,e1")
        self.phases = tuple(x for x in _ph.split(",") if x)


def _balance_windows(cfg, deg_lo, deg_hi):
    """Greedy 2D bin pack: nodes -> (window, pos) on one core, keeping each
    window's low-row AND high-row edge counts within chunk capacity."""
    NW, W = cfg.NW, cfg.W
    order_n = np.argsort(-(deg_lo + deg_hi), kind="stable")
    loads_l = np.zeros(NW, np.int64)
    loads_h = np.zeros(NW, np.int64)
    counts = np.zeros(NW, np.int64)
    w_of = np.zeros(len(order_n), np.int64)
    p_of = np.zeros(len(order_n), np.int64)
    for n in order_n:
        dl, dh = deg_lo[n], deg_hi[n]
        # least max-fill window (normalized headroom) among non-full windows
        free = counts < W
        score = np.maximum((loads_l + dl) / cfg.CAPL, (loads_h + dh) / cfg.CAPH)
        score[~free] = np.inf
        w = int(np.argmin(score))
        w_of[n] = w
        p_of[n] = counts[w]
        counts[w] += 1
        loads_l[w] += dl
        loads_h[w] += dh
    assert loads_l.max() <= cfg.CAPL and loads_h.max() <= cfg.CAPH, (
        f"window overflow: lo {loads_l.max()}/{cfg.CAPL} "
        f"hi {loads_h.max()}/{cfg.CAPH}; raise J/NWX")
    return w_of, p_of


def preprocess(cfg, edge_index, edge_attr):
    """Pack edges into degree-balanced 128-dst-node windows, per core.

    Window slot layout: low-table rows (src xl row < HALF) fill slots
    [0:n_lo] (chunks 0..NCH/2), high-table rows fill [SW/2 : SW/2+n_hi]
    (chunks NCH/2..NCH). Each chunk is one dma_gather with trailing -1
    indices (trimmed by the Q7 kernel), so descriptors == real edges and
    no register bookkeeping is needed. Node->(window,pos) assignment is a
    2D greedy bin-pack over (low, high) incoming degree; `rows` maps each
    local node to its permuted row for x_T/output (un)permutation.
    """
    src = np.asarray(edge_index[0], np.int64)
    dst = np.asarray(edge_index[1], np.int64)
    ea = np.asarray(edge_attr, np.float32)
    NV, W, NW, SW, SWC = cfg.NV, cfg.W, cfg.NW, cfg.SW, cfg.SWC
    EF = ea.shape[1]
    HSW, NCH = cfg.HSW, cfg.NCH

    # low rows come from the first NC/2 cores (xl row < HALF), fixed by the
    # natural core of each src node — independent of the permutation
    is_lo_e = src < (cfg.N // 2)
    deg_lo_all = np.bincount(dst[is_lo_e], minlength=cfg.N)
    deg_hi_all = np.bincount(dst[~is_lo_e], minlength=cfg.N)
    w_of = np.zeros(cfg.N, np.int64)
    p_of = np.zeros(cfg.N, np.int64)
    for c in range(cfg.NC):
        sl = slice(c * NV, (c + 1) * NV)
        w_of[sl], p_of[sl] = _balance_windows(
            cfg, deg_lo_all[sl], deg_hi_all[sl])

    # permuted-global row of each node's xl entry
    core_of = src // NV
    grow = core_of * cfg.NVP + w_of[src] * W + p_of[src]

    order = np.argsort(dst, kind="stable")
    wrap = lambda a: np.tile(a.reshape(len(a) // 16, 16).T, (8, 1))
    cores = []
    for c in range(cfg.NC):
        lo = np.searchsorted(dst, c * NV, side="left", sorter=order)
        hi = np.searchsorted(dst, (c + 1) * NV, side="left", sorter=order)
        eidx_c = order[lo:hi]
        win = w_of[dst[eidx_c]]
        pos = p_of[dst[eidx_c]]

        idx2 = np.zeros((NW, 128, NCH, HSW // 16), np.int16)
        dstf = np.zeros((NW, 128, SWC), np.float16)
        eat = np.zeros((NW, EF + 1, SW), np.float16)

        for w in range(NW):
            mask = win == w
            e_w = eidx_c[mask]
            g_w = grow[e_w]
            dl_w = pos[mask]
            o = np.argsort(g_w, kind="stable")
            e_w, g_w, dl_w = e_w[o], g_w[o], dl_w[o]
            n_lo = int((g_w < cfg.HALF).sum())
            n_hi = len(e_w) - n_lo
            assert n_lo <= cfg.CAPL and n_hi <= cfg.CAPH

            drow = np.full(SW, -1.0, np.float16)
            eaT = np.zeros((EF, SW), np.float16)
            dl = dl_w.astype(np.float16)
            h0 = SW // 2                   # first high-row slot
            drow[:n_lo] = dl[:n_lo]
            drow[h0:h0 + n_hi] = dl[n_lo:]
            eaT[:, :n_lo] = ea[e_w[:n_lo]].T
            eaT[:, h0:h0 + n_hi] = ea[e_w[n_lo:]].T
            # pad slots gather row 0 of their table: descriptor count stays
            # equal to the decode-side reservation (a trailing -1 trim would
            # desync the SWDGE ring bookkeeping and wedge the device), and
            # every stage slot gets written with finite data.
            idxs = np.zeros(SW, np.int16)
            idxs[:n_lo] = g_w[:n_lo].astype(np.int16)
            idxs[h0:h0 + n_hi] = (g_w[n_lo:] - cfg.HALF).astype(np.int16)
            for q in range(NCH):
                idx2[w, :, q, :] = wrap(idxs[q * HSW:(q + 1) * HSW])
            dstf[w] = drow.reshape(SWC, 128).T
            eat[w, EF, :] = drow
            eat[w, :EF, :] = eaT
        rows = w_of[c * NV:(c + 1) * NV] * W + p_of[c * NV:(c + 1) * NV]
        cores.append(dict(idx2=idx2, dstf=dstf, eat=eat, rows=rows))
    return cores


def build_program(cfg, nc):
    """Emit the full SPMD program into nc (a Bacc) under TileContext."""
    F, G, T_E, J, NW, SW, SWC, NVP, W = (cfg.F, cfg.G, cfg.T_E, cfg.J, cfg.NW,
                                         cfg.SW, cfg.SWC, cfg.NVP, cfg.W)
    EF = 32
    P = {}
    def inp(name, shape, dt):
        P[name] = nc.dram_tensor(name, shape, dt, kind="ExternalInput")
        return P[name]

    x_T = inp("x_T", [cfg.IN_F, NVP], F16)
    idx2 = inp("idx2", [NW, 128, cfg.NCH, cfg.HSW // 16], I16)
    dstf = inp("dstf", [NW, 128, SWC], F16)
    eat = inp("eat", [NW, EF + 1, SW], F16)
    wl0 = inp("wl0", [cfg.IN_F, F], F16)
    wr0 = inp("wr0", [cfg.IN_F, F], F16)
    we0 = inp("we0", [EF, F], F16)
    wl1 = inp("wl1", [F, F], F16)
    wr1 = inp("wr1", [F, F], F16)
    we1 = inp("we1", [EF, F], F16)
    wout = inp("wout", [128, 2, 1], F16)
    att0 = inp("att0", [128, 2, 4], F16)
    att1 = inp("att1", [128, 2, 4], F16)
    bl0 = inp("bl0", [128, F], F32)
    br0 = inp("br0", [128, F], F32)
    bias0 = inp("bias0", [128, F], F32)
    bl1 = inp("bl1", [128, F], F32)
    br1 = inp("br1", [128, F], F32)
    bias1 = inp("bias1", [128, F], F32)
    bout = inp("bout", [128, 1], F32)
    iota_r16 = inp("iota_r16", [128, 128], F16)
    iota_c = inp("iota_c", [128, 1], F32)
    ones16 = inp("ones16", [33, 128], F16)
    ident_in = inp("ident_in", [128, 128], F16)

    out_own = nc.dram_tensor("out_own", [NVP, 1], F32, kind="ExternalOutput")

    # ---- internal DRAM
    xl0_own = nc.dram_tensor("xl0_own", [NVP, F], F16)
    xl1_own = nc.dram_tensor("xl1_own", [NVP, F], F16)
    akw = dict(addr_space="Shared") if cfg.NC > 4 else {}
    xl0_full = nc.dram_tensor("xl0_full", [cfg.NFULL, F], F16, **akw)
    xl1_full = nc.dram_tensor("xl1_full", [cfg.NFULL, F], F16, **akw)
    groups = [list(range(cfg.NC))]

    with tile.TileContext(nc) as tc:
        with (
            tc.tile_pool(name="const", bufs=1) as constp,
            tc.tile_pool(name="wpool", bufs=1) as wpool,
            tc.tile_pool(name="resid", bufs=1) as resid,
            tc.tile_pool(name="io", bufs=3) as io,
            tc.tile_pool(name="stg", bufs=2) as stg,
            tc.tile_pool(name="sel", bufs=3) as selp,
            tc.tile_pool(name="mpool", bufs=2) as mpool,
            tc.tile_pool(name="small", bufs=4) as small,
            tc.tile_pool(name="psA", bufs=2, space="PSUM") as psA,
            tc.tile_pool(name="psW", bufs=2, space="PSUM") as psW,
            tc.tile_pool(name="psS", bufs=2, space="PSUM") as psS,
        ):
            ident16 = constp.tile([128, 128], F16)
            nc.sync.dma_start(out=ident16[:], in_=ident_in[:])
            iota_row = constp.tile([128, 128], F16)
            nc.sync.dma_start(out=iota_row[:], in_=iota_r16[:])
            iota_col = constp.tile([128, 1], F32)
            nc.sync.dma_start(out=iota_col[:], in_=iota_c[:])
            ones_sb = constp.tile([33, 128], F16)
            nc.sync.dma_start(out=ones_sb[:], in_=ones16[:])
            batt = {}
            for nm, t in (("att0", att0), ("att1", att1)):
                bt = constp.tile([128, 2, 4], F16, tag=nm)
                nc.sync.dma_start(out=bt[:], in_=t[:])
                batt[nm] = bt
            bout_sb = constp.tile([128, 1], F32)
            nc.sync.dma_start(out=bout_sb[:], in_=bout[:])
            bsb = {}
            for nm, t in (("bl0", bl0), ("br0", br0), ("bias0", bias0),
                          ("bl1", bl1), ("br1", br1), ("bias1", bias1)):
                bt = constp.tile([128, F], F32, tag=nm)
                nc.sync.dma_start(out=bt[:], in_=t[:])
                bsb[nm] = bt

            def load_w(t, kdim, tag):
                n = kdim // 128
                w = wpool.tile([128, n, F], F16, tag=tag)
                nc.sync.dma_start(
                    out=w[:], in_=t.rearrange("(k p) f -> p k f", p=128))
                return w
            wl0_sb = load_w(wl0, cfg.IN_F, "wl0")
            wr0_sb = load_w(wr0, cfg.IN_F, "wr0")
            wl1_sb = load_w(wl1, F, "wl1")
            wr1_sb = load_w(wr1, F, "wr1")
            we0_sb = wpool.tile([EF, F], F16, tag="we0")
            nc.sync.dma_start(out=we0_sb[:], in_=we0[:])
            we1_sb = wpool.tile([EF, F], F16, tag="we1")
            nc.sync.dma_start(out=we1_sb[:], in_=we1[:])
            wout_sb = wpool.tile([128, 2, 1], F16, tag="wout")
            nc.sync.dma_start(out=wout_sb[:], in_=wout[:])

            # SBUF-resident xr (shared by both layers) and inter-layer h1
            xr_sb = resid.tile([128, NW, F], F16, tag="xr")
            h1_sb = resid.tile([128, NW, F], F16, tag="h1")

            # ---------------- layer-0 projections ----------------
            def phase_p1():
                for c in range(NW):
                    xk = io.tile([128, cfg.KCH, 128], F16, tag="xk")
                    nc.sync.dma_start(
                        out=xk[:],
                        in_=x_T.rearrange("(k p) n -> p k n", p=128)
                            [:, :, c * 128:(c + 1) * 128])
                    plr = psA.tile([128, 2, F], F32, tag="pm", bufs=2)
                    for k in range(cfg.KCH):
                        nc.tensor.matmul(plr[:, 0, :], lhsT=xk[:, k, :],
                                         rhs=wl0_sb[:, k, :],
                                         start=(k == 0), stop=(k == cfg.KCH - 1),
                                         skip_group_check=True)
                    for k in range(cfg.KCH):
                        nc.tensor.matmul(plr[:, 1, :], lhsT=xk[:, k, :],
                                         rhs=wr0_sb[:, k, :],
                                         start=(k == 0), stop=(k == cfg.KCH - 1),
                                         skip_group_check=True)
                    ol = io.tile([128, F], F16, tag="oxl")
                    nc.vector.tensor_add(out=ol[:], in0=plr[:, 0, :],
                                         in1=bsb["bl0"][:])
                    nc.vector.tensor_add(out=xr_sb[:, c, :], in0=plr[:, 1, :],
                                         in1=bsb["br0"][:])
                    nc.sync.dma_start(out=xl0_own[c * 128:(c + 1) * 128, :],
                                      in_=ol[:])

            # ---------------- edge pass (shared for both layers) ------------
            def edge_pass(layer, xl_full, we_sb, att_sb, bias_sb, cc=None,
                          tail=None):
                NCH = cfg.NCH
                HC = SWC // NCH
                for w in range(NW):
                    idx_sb = io.tile([128, NCH, cfg.HSW // 16], I16,
                                     tag="idx")
                    nc.sync.dma_start(out=idx_sb[:], in_=idx2[w])
                    ea_sb = io.tile([EF + 1, SW], F16, tag="ea")
                    nc.sync.dma_start(out=ea_sb[:], in_=eat[w])
                    dstf_sb = io.tile([128, SWC], F16, tag="dstf")
                    nc.sync.dma_start(out=dstf_sb[:], in_=dstf[w])
                    stage = stg.tile([128, SWC, F], F16, tag="stage")
                    for q in range(NCH):
                        # chunks 0..NCH/2 gather low table rows; rest high.
                        tbl = (xl_full[:cfg.HALF, :] if q < NCH // 2
                               else xl_full[cfg.HALF:, :])
                        g1 = nc.gpsimd.dma_gather(
                            out_ap=stage[:, q * HC:(q + 1) * HC, :],
                            in_ap=tbl,
                            idxs_ap=idx_sb[:, q, :], num_idxs=cfg.HSW,
                            num_idxs_reg=cfg.HSW, elem_size=F)
                        if cc is not None:
                            bass._add_dep_helper(
                                g1.ins, cc.ins, sync=True,
                                reason="gather reads AllGather output")
                    # pall: [0:256] weighted sum | [256:260] denominator
                    pall = psW.tile([128, 260], F32, tag="pall")
                    for j in range(J):
                        ed = ea_sb[:, j * T_E:(j + 1) * T_E]
                        drow = psS.tile([128, T_E], F32, tag="tmp")
                        nc.tensor.matmul(drow[:], lhsT=ones_sb[EF:EF + 1, :],
                                         rhs=ed[EF:EF + 1, :],
                                         start=True, stop=True,
                                         skip_group_check=True)
                        st_j = selp.tile([128, T_E], F16, tag="st")
                        nc.vector.tensor_tensor(
                            out=st_j[:],
                            in0=iota_col[:].to_broadcast([128, T_E]),
                            in1=drow[:], op=mybir.AluOpType.is_equal)
                        s_j = selp.tile([128, G, 128], F16, tag="s")
                        nc.vector.tensor_tensor(
                            out=s_j[:],
                            in0=dstf_sb[:, G * j:G * j + G]
                                .unsqueeze(-1).to_broadcast([128, G, 128]),
                            in1=iota_row[:].unsqueeze(1)
                                .to_broadcast([128, G, 128]),
                            op=mybir.AluOpType.is_equal)
                        m_t = mpool.tile([128, 2, T_E], F16, tag="m")
                        for h in range(2):
                            pm = psA.tile([128, T_E], F32, tag="pm", bufs=2)
                            nc.tensor.matmul(
                                pm[:], lhsT=we_sb[:, h * 128:(h + 1) * 128],
                                rhs=ed[:EF, :], start=True, stop=False,
                                skip_group_check=True)
                            nc.tensor.matmul(
                                pm[:],
                                lhsT=xr_sb[:, w, h * 128:(h + 1) * 128],
                                rhs=st_j[:], start=False, stop=False,
                                skip_group_check=True)
                            # transpose-accumulate gathered xl[src] via
                            # identity matmul (stage^T @ I), f32 accumulate
                            for g in range(G):
                                nc.tensor.matmul(
                                    pm[:, g * 128:(g + 1) * 128],
                                    lhsT=stage[:, G * j + g,
                                               h * 128:(h + 1) * 128],
                                    rhs=ident16[:],
                                    start=False, stop=(g == G - 1),
                                    skip_group_check=True)
                            rp = mpool.tile([128, T_E], F16, tag="rp")
                            nc.scalar.activation(
                                rp[:], pm[:], mybir.ActivationFunctionType.Relu,
                                scale=1.0 - NEG_SLOPE)
                            nc.vector.scalar_tensor_tensor(
                                out=m_t[:, h, :], in0=pm[:], scalar=NEG_SLOPE,
                                in1=rp[:], op0=mybir.AluOpType.mult,
                                op1=mybir.AluOpType.add)
                        plog = psS.tile([128, 16], F32, tag="tmp")
                        for g in range(G):
                            for h in range(2):
                                nc.tensor.matmul(
                                    plog[:, 4 * g:4 * g + 4],
                                    lhsT=m_t[:, h, g * 128:(g + 1) * 128],
                                    rhs=att_sb[:, h, :],
                                    start=(h == 0), stop=(h == 1),
                                    skip_group_check=True)
                        v = small.tile([128, G, 260], F16, tag="v")
                        nc.scalar.activation(
                            v[:, :, 256:260],
                            plog[:].rearrange("p (g h) -> p g h", g=4),
                            mybir.ActivationFunctionType.Exp)
                        nc.vector.tensor_tensor(
                            out=v[:, :, :256].rearrange(
                                "p g (h c) -> p g h c", h=4),
                            in0=stage[:, G * j:G * (j + 1), :].rearrange(
                                "p g (h c) -> p g h c", h=4),
                            in1=v[:, :, 256:260]
                                .unsqueeze(-1).to_broadcast([128, G, 4, 64]),
                            op=mybir.AluOpType.mult)
                        for g in range(G):
                            nc.tensor.matmul(
                                pall[:, 0:260], lhsT=s_j[:, g, :],
                                rhs=v[:, g, :],
                                start=(j == 0 and g == 0),
                                stop=(j == J - 1 and g == G - 1),
                                skip_group_check=True)
                    rdf = small.tile([128, 4], F32, tag="rdf")
                    nc.vector.tensor_scalar_add(out=rdf[:],
                                                in0=pall[:, 256:260],
                                                scalar1=EPS)
                    rden = small.tile([128, 4], F32, tag="rden")
                    nc.vector.reciprocal(out=rden[:], in_=rdf[:])
                    hs = small.tile([128, F], F32, tag="hs")
                    nc.vector.tensor_tensor(
                        out=hs.rearrange("p (h c) -> p h c", h=4),
                        in0=pall[:, 0:256].rearrange("p (h c) -> p h c", h=4),
                        in1=rden.unsqueeze(-1).to_broadcast([128, 4, 64]),
                        op=mybir.AluOpType.mult)
                    hb = small.tile([128, F], F32, tag="hb")
                    nc.vector.tensor_add(out=hb[:], in0=hs[:], in1=bias_sb[:])
                    if layer == 0:
                        nc.scalar.activation(h1_sb[:, w, :], hb[:],
                                             mybir.ActivationFunctionType.Relu)
                        if tail is not None:
                            tail(w)
                    else:
                        h_out = small.tile([128, F], F16, tag="hout")
                        nc.scalar.activation(h_out[:], hb[:],
                                             mybir.ActivationFunctionType.Relu)
                        po = psS.tile([128, 1], F32, tag="tmp")
                        for h in range(2):
                            pt = psS.tile([128, 128], F16, tag="tmp")
                            nc.tensor.matmul(pt[:],
                                             lhsT=h_out[:, h * 128:(h + 1) * 128],
                                             rhs=ident16[:], is_transpose=True,
                                             start=True, stop=True,
                                             skip_group_check=True)
                            h2T = small.tile([128, 128], F16, tag="h2T")
                            nc.vector.tensor_copy(out=h2T[:], in_=pt[:])
                            nc.tensor.matmul(po[:], lhsT=h2T[:],
                                             rhs=wout_sb[:, h, :],
                                             start=(h == 0), stop=(h == 1),
                                             skip_group_check=True)
                        o_sb = small.tile([128, 1], F32, tag="osb")
                        nc.vector.tensor_scalar(
                            out=o_sb[:], in0=po[:], scalar1=bout_sb[:, :1],
                            scalar2=None, op0=mybir.AluOpType.add)
                        nc.sync.dma_start(out=out_own[w * W:(w + 1) * W, :],
                                          in_=o_sb[:])

            # ---------------- layer-1 projections ----------------
            def p4_chunk(c):
                h1T = io.tile([128, 2, 128], F16, tag="h1T")
                for h in range(2):
                    pt = psS.tile([128, 128], F16, tag="tmp")
                    nc.tensor.matmul(pt[:],
                                     lhsT=h1_sb[:, c, h * 128:(h + 1) * 128],
                                     rhs=ident16[:], is_transpose=True,
                                     start=True, stop=True,
                                     skip_group_check=True)
                    nc.vector.tensor_copy(out=h1T[:, h, :], in_=pt[:])
                plr = psA.tile([128, 2, F], F32, tag="pm", bufs=2)
                for h in range(2):
                    nc.tensor.matmul(plr[:, 0, :], lhsT=h1T[:, h, :],
                                     rhs=wl1_sb[:, h, :],
                                     start=(h == 0), stop=(h == 1),
                                     skip_group_check=True)
                for h in range(2):
                    nc.tensor.matmul(plr[:, 1, :], lhsT=h1T[:, h, :],
                                     rhs=wr1_sb[:, h, :],
                                     start=(h == 0), stop=(h == 1),
                                     skip_group_check=True)
                ol = io.tile([128, F], F16, tag="oxl")
                nc.vector.tensor_add(out=ol[:], in0=plr[:, 0, :],
                                     in1=bsb["bl1"][:])
                nc.vector.tensor_add(out=xr_sb[:, c, :], in0=plr[:, 1, :],
                                     in1=bsb["br1"][:])
                nc.sync.dma_start(out=xl1_own[c * 128:(c + 1) * 128, :],
                                  in_=ol[:])

            def phase_p4():
                for c in range(NW):
                    p4_chunk(c)

            def phase_ag(xl_own, xl_full):
                # Pre-barrier ensures the xl shard DMA writes landed before
                # the collective reads them. No post-AG barrier: the only
                # consumers of xl_full are the dma_gathers, which carry an
                # explicit dep on the collective — window preambles (loads,
                # selector builds) overlap the collective.
                tc.strict_bb_all_engine_barrier()
                cc = nc.gpsimd.collective_compute(
                    "AllGather", mybir.AluOpType.bypass, replica_groups=groups,
                    ins=[xl_own[:]], outs=[xl_full[:]])
                return cc

            for _rep in range(cfg.reps):
                cc0 = cc1 = None
                if "p1" in cfg.phases:
                    phase_p1()
                if "ag0" in cfg.phases:
                    cc0 = phase_ag(xl0_own, xl0_full)
                fuse_p4 = "p4" in cfg.phases and "e0" in cfg.phases
                if "e0" in cfg.phases:
                    # layer-1 projection chunks run inside the layer-0 edge
                    # pass (chunk c right after window c's h1 is ready), so
                    # xl1 is complete the moment e0 ends and ag1 starts
                    # immediately.
                    edge_pass(0, xl0_full, we0_sb, batt["att0"], bsb["bias0"],
                              cc=cc0, tail=(p4_chunk if fuse_p4 else None))
                if "p4" in cfg.phases and not fuse_p4:
                    tc.strict_bb_all_engine_barrier()
                    phase_p4()
                if "ag1" in cfg.phases:
                    cc1 = phase_ag(xl1_own, xl1_full)
                if "e1" in cfg.phases:
                    edge_pass(1, xl1_full, we1_sb, batt["att1"], bsb["bias1"],
                              cc=cc1)
                # Rep-end barrier: guarantees the collectives' reads and all
                # in-flight gather DMAs completed before the next rep (or
                # program end) rewrites their sources.
                tc.strict_bb_all_engine_barrier()
    return P


_CACHE = {}


def _get_compiled(cfg):
    key = (cfg.N, cfg.E, cfg.IN_F, cfg.NC, cfg.J, cfg.reps, cfg.phases,
           cfg.NWX)
    if key not in _CACHE:
        import os as _os
        nc = bacc.Bacc("TRN2", target_bir_lowering=False, debug=False,
                       num_devices=cfg.NC,
                       dynamic_dma_scratch_size=int(
                           _os.environ.get("K_SCRATCH", "49152")),
                       num_swdge_queues=int(_os.environ.get("K_QUEUES", "1")))
        build_program(cfg, nc)
        nc.compile()
        _CACHE[key] = nc
    return _CACHE[key]


def make_in_maps(cfg, inputs, cores_pre):
    """Per-core input dicts."""
    x = np.asarray(inputs["x"], np.float32)
    H, C, F = cfg.H, cfg.C, cfg.F
    att_blk = {}
    for li in (0, 1):
        att = np.asarray(inputs[f"att{li}"], np.float32)   # [H, C]
        A = np.zeros((2 * 128, 4), np.float32)
        for h in range(H):
            A[h * C:(h + 1) * C, h] = att[h]
        att_blk[li] = np.ascontiguousarray(
            A.reshape(2, 128, 4).transpose(1, 0, 2)).astype(np.float16)
    iota_r16 = np.tile(np.arange(128, dtype=np.float16)[None, :], (128, 1))
    iota_c = np.arange(128, dtype=np.float32).reshape(128, 1)
    ones16 = np.ones((33, 128), np.float16)
    rep = lambda v: np.tile(np.asarray(v, np.float32)[None, :], (128, 1))
    f16 = lambda v: np.asarray(v, np.float32).astype(np.float16)
    common = dict(
        wl0=f16(inputs["W_l0"]), wr0=f16(inputs["W_r0"]),
        we0=f16(inputs["W_e0"]), wl1=f16(inputs["W_l1"]),
        wr1=f16(inputs["W_r1"]), we1=f16(inputs["W_e1"]),
        wout=f16(inputs["W_out"]).reshape(2, 128, 1).transpose(1, 0, 2).copy(),
        att0=att_blk[0], att1=att_blk[1],
        bl0=rep(inputs["b_l0"]), br0=rep(inputs["b_r0"]),
        bias0=rep(inputs["bias0"]), bl1=rep(inputs["b_l1"]),
        br1=rep(inputs["b_r1"]), bias1=rep(inputs["bias1"]),
        bout=np.tile(np.asarray(inputs["b_out"], np.float32).reshape(1, 1),
                     (128, 1)),
        iota_r16=iota_r16, iota_c=iota_c, ones16=ones16,
        ident_in=np.eye(128, dtype=np.float16),
    )
    in_maps = []
    for c in range(cfg.NC):
        pre = cores_pre[c]
        xs = np.zeros((cfg.NVP, cfg.IN_F), np.float32)
        xs[pre["rows"]] = x[c * cfg.NV:(c + 1) * cfg.NV]
        m = dict(common)
        m.update(x_T=np.ascontiguousarray(xs.T).astype(np.float16),
                 idx2=pre["idx2"], dstf=pre["dstf"],
                 eat=pre["eat"])
        in_maps.append(m)
    return in_maps


def _run(cfg, inputs):
    cores_pre = preprocess(cfg, inputs["edge_index"], inputs["edge_attr"])
    in_maps = make_in_maps(cfg, inputs, cores_pre)
    nc = _get_compiled(cfg)
    res = run_bass_kernel_spmd(nc, in_maps, core_ids=list(range(cfg.NC)))
    outs = []
    for c in range(cfg.NC):
        outs.append(res.results[c]["out_own"][cores_pre[c]["rows"]])
    return np.concatenate(outs, 0).astype(np.float32)


def pick_cfg(edge_index, reps=1):
    """Smallest (J, NWX) whose 2D-balanced packing fits this graph."""
    src = np.asarray(edge_index[0], np.int64)
    dst = np.asarray(edge_index[1], np.int64)
    for J, NWX in ((4, 1), (5, 1), (5, 2), (6, 2), (8, 4)):
        cfg = Cfg(N=50000, E=800000, IN_F=512, NC=8, J=J, reps=reps, NWX=NWX)
        is_lo = src < (cfg.N // 2)
        deg_lo = np.bincount(dst[is_lo], minlength=cfg.N)
        deg_hi = np.bincount(dst[~is_lo], minlength=cfg.N)
        ok = True
        for c in range(cfg.NC):
            sl = slice(c * cfg.NV, (c + 1) * cfg.NV)
            try:
                _balance_windows(cfg, deg_lo[sl], deg_hi[sl])
            except AssertionError:
                ok = False
                break
        if ok:
            return cfg
    return cfg


def kernel(**inputs):
    cfg = pick_cfg(inputs["edge_index"])
    return _run(cfg, inputs)



# revision 21
# speedup vs baseline: 1.1009x; 1.1009x over previous
"""Trainium2 Bass kernel for 2-layer GATv2 + output projection (SPMD, 8 cores).

v2 strategy: nodes partitioned across cores; per 128-dst-node window, edges
are packed into SW=J*512 slots (sorted by gathered-row id so the low/high
int16 gather split is two window-level dma_gather calls). fp16 data plane
(same mantissa as TF32/f32r), f32 PSUM accumulation. Softmax denominator is
factored out of the weighted scatter (out = (sum ex*xl[src]) / denom), so
each window is a single pass: per 512-slot tile, edge-feature projection +
xr[dst] broadcast (selector matmul) + transposed gathered xl accumulate into
PSUM; fused leaky-relu; per-head logits via PE; exp on ACT; one selector
matmul per 128-slot group scatters both the weighted values and the
denominator (values carry ex in 4 extra columns). Halo exchange = AllGather
of fp16 xl shards. xr and the inter-layer h1 stay SBUF-resident.
"""
import numpy as np

import concourse.bass as bass
import concourse.bacc as bacc
import concourse.mybir as mybir
import concourse.tile as tile
from concourse.bass_utils import run_bass_kernel_spmd
from concourse.masks import make_identity

F32 = mybir.dt.float32
F16 = mybir.dt.float16
I16 = mybir.dt.int16
I32 = mybir.dt.int32

NEG_SLOPE = 0.2
EPS = 1e-30


class Cfg:
    def __init__(self, N, E, IN_F, NC, J, reps=1, NWX=0):
        self.N, self.E, self.IN_F, self.NC, self.J = N, E, IN_F, NC, J
        self.reps = reps
        self.NWX = NWX
        self.F = 256
        self.H, self.C = 4, 64
        self.W = 128                      # dst nodes per window
        assert N % NC == 0
        self.NV = N // NC                 # nodes per core
        self.NW = (self.NV + self.W - 1) // self.W + NWX
        self.NVP = self.NW * self.W       # padded nodes per core
        self.T_E = 512                    # edge slots per tile
        self.G = 4                        # 128-edge groups per tile
        self.SW = self.J * self.T_E       # edge slots per window
        self.SWC = self.SW // 128         # 128-slot chunks per window
        self.NCH = 4                      # gather chunks/window: 2 low, 2 high
        assert self.SW % self.NCH == 0
        self.HSW = self.SW // self.NCH    # slots per gather chunk
        assert self.HSW % 16 == 0 and (self.HSW // 128) * 128 == self.HSW
        self.CAPL = 2 * self.HSW          # low-row slot capacity per window
        self.CAPH = 2 * self.HSW          # high-row slot capacity per window
        self.NFULL = self.NC * self.NVP   # rows in gathered xl table
        assert self.NFULL % 2 == 0
        self.HALF = self.NFULL // 2
        assert self.HALF <= 32768, "int16 gather index range"
        self.KCH = self.IN_F // 128       # K-chunks for layer-0 projection
        import os
        _ph = os.environ.get("K_PHASES", "p1,ag0,e0,p4,ag1,e1")
        self.phases = tuple(x for x in _ph.split(",") if x)


def _balance_windows(cfg, deg_lo, deg_hi):
    """Greedy 2D bin pack: nodes -> (window, pos) on one core, keeping each
    window's low-row AND high-row edge counts within chunk capacity."""
    NW, W = cfg.NW, cfg.W
    order_n = np.argsort(-(deg_lo + deg_hi), kind="stable")
    loads_l = np.zeros(NW, np.int64)
    loads_h = np.zeros(NW, np.int64)
    counts = np.zeros(NW, np.int64)
    w_of = np.zeros(len(order_n), np.int64)
    p_of = np.zeros(len(order_n), np.int64)
    for n in order_n:
        dl, dh = deg_lo[n], deg_hi[n]
        # least max-fill window (normalized headroom) among non-full windows
        free = counts < W
        score = np.maximum((loads_l + dl) / cfg.CAPL, (loads_h + dh) / cfg.CAPH)
        score[~free] = np.inf
        w = int(np.argmin(score))
        w_of[n] = w
        p_of[n] = counts[w]
        counts[w] += 1
        loads_l[w] += dl
        loads_h[w] += dh
    assert loads_l.max() <= cfg.CAPL and loads_h.max() <= cfg.CAPH, (
        f"window overflow: lo {loads_l.max()}/{cfg.CAPL} "
        f"hi {loads_h.max()}/{cfg.CAPH}; raise J/NWX")
    return w_of, p_of


def preprocess(cfg, edge_index, edge_attr):
    """Pack edges into degree-balanced 128-dst-node windows, per core.

    Window slot layout: low-table rows (src xl row < HALF) fill slots
    [0:n_lo] (chunks 0..NCH/2), high-table rows fill [SW/2 : SW/2+n_hi]
    (chunks NCH/2..NCH). Each chunk is one dma_gather with trailing -1
    indices (trimmed by the Q7 kernel), so descriptors == real edges and
    no register bookkeeping is needed. Node->(window,pos) assignment is a
    2D greedy bin-pack over (low, high) incoming degree; `rows` maps each
    local node to its permuted row for x_T/output (un)permutation.
    """
    src = np.asarray(edge_index[0], np.int64)
    dst = np.asarray(edge_index[1], np.int64)
    ea = np.asarray(edge_attr, np.float32)
    NV, W, NW, SW, SWC = cfg.NV, cfg.W, cfg.NW, cfg.SW, cfg.SWC
    EF = ea.shape[1]
    HSW, NCH = cfg.HSW, cfg.NCH

    # low rows come from the first NC/2 cores (xl row < HALF), fixed by the
    # natural core of each src node — independent of the permutation
    is_lo_e = src < (cfg.N // 2)
    deg_lo_all = np.bincount(dst[is_lo_e], minlength=cfg.N)
    deg_hi_all = np.bincount(dst[~is_lo_e], minlength=cfg.N)
    w_of = np.zeros(cfg.N, np.int64)
    p_of = np.zeros(cfg.N, np.int64)
    for c in range(cfg.NC):
        sl = slice(c * NV, (c + 1) * NV)
        w_of[sl], p_of[sl] = _balance_windows(
            cfg, deg_lo_all[sl], deg_hi_all[sl])

    # permuted-global row of each node's xl entry
    core_of = src // NV
    grow = core_of * cfg.NVP + w_of[src] * W + p_of[src]

    order = np.argsort(dst, kind="stable")
    wrap = lambda a: np.tile(a.reshape(len(a) // 16, 16).T, (8, 1))
    cores = []
    for c in range(cfg.NC):
        lo = np.searchsorted(dst, c * NV, side="left", sorter=order)
        hi = np.searchsorted(dst, (c + 1) * NV, side="left", sorter=order)
        eidx_c = order[lo:hi]
        win = w_of[dst[eidx_c]]
        pos = p_of[dst[eidx_c]]

        idx2 = np.zeros((NW, 128, NCH, HSW // 16), np.int16)
        dstf = np.zeros((NW, 128, SWC), np.float16)
        eat = np.zeros((NW, EF + 1, SW), np.float16)

        for w in range(NW):
            mask = win == w
            e_w = eidx_c[mask]
            g_w = grow[e_w]
            dl_w = pos[mask]
            o = np.argsort(g_w, kind="stable")
            e_w, g_w, dl_w = e_w[o], g_w[o], dl_w[o]
            n_lo = int((g_w < cfg.HALF).sum())
            n_hi = len(e_w) - n_lo
            assert n_lo <= cfg.CAPL and n_hi <= cfg.CAPH

            drow = np.full(SW, -1.0, np.float16)
            eaT = np.zeros((EF, SW), np.float16)
            dl = dl_w.astype(np.float16)
            h0 = SW // 2                   # first high-row slot
            drow[:n_lo] = dl[:n_lo]
            drow[h0:h0 + n_hi] = dl[n_lo:]
            eaT[:, :n_lo] = ea[e_w[:n_lo]].T
            eaT[:, h0:h0 + n_hi] = ea[e_w[n_lo:]].T
            # pad slots gather row 0 of their table: descriptor count stays
            # equal to the decode-side reservation (a trailing -1 trim would
            # desync the SWDGE ring bookkeeping and wedge the device), and
            # every stage slot gets written with finite data.
            idxs = np.zeros(SW, np.int16)
            idxs[:n_lo] = g_w[:n_lo].astype(np.int16)
            idxs[h0:h0 + n_hi] = (g_w[n_lo:] - cfg.HALF).astype(np.int16)
            for q in range(NCH):
                idx2[w, :, q, :] = wrap(idxs[q * HSW:(q + 1) * HSW])
            dstf[w] = drow.reshape(SWC, 128).T
            eat[w, EF, :] = drow
            eat[w, :EF, :] = eaT
        rows = w_of[c * NV:(c + 1) * NV] * W + p_of[c * NV:(c + 1) * NV]
        cores.append(dict(idx2=idx2, dstf=dstf, eat=eat, rows=rows))
    return cores


def build_program(cfg, nc):
    """Emit the full SPMD program into nc (a Bacc) under TileContext."""
    F, G, T_E, J, NW, SW, SWC, NVP, W = (cfg.F, cfg.G, cfg.T_E, cfg.J, cfg.NW,
                                         cfg.SW, cfg.SWC, cfg.NVP, cfg.W)
    EF = 32
    P = {}
    def inp(name, shape, dt):
        P[name] = nc.dram_tensor(name, shape, dt, kind="ExternalInput")
        return P[name]

    x_T = inp("x_T", [cfg.IN_F, NVP], F16)
    idx2 = inp("idx2", [NW, 128, cfg.NCH, cfg.HSW // 16], I16)
    dstf = inp("dstf", [NW, 128, SWC], F16)
    eat = inp("eat", [NW, EF + 1, SW], F16)
    wl0 = inp("wl0", [cfg.IN_F, F], F16)
    wr0 = inp("wr0", [cfg.IN_F, F], F16)
    we0 = inp("we0", [EF, F], F16)
    wl1 = inp("wl1", [F, F], F16)
    wr1 = inp("wr1", [F, F], F16)
    we1 = inp("we1", [EF, F], F16)
    wout = inp("wout", [128, 2, 1], F16)
    att0 = inp("att0", [128, 2, 4], F16)
    att1 = inp("att1", [128, 2, 4], F16)
    bl0 = inp("bl0", [128, F], F32)
    br0 = inp("br0", [128, F], F32)
    bias0 = inp("bias0", [128, F], F32)
    bl1 = inp("bl1", [128, F], F32)
    br1 = inp("br1", [128, F], F32)
    bias1 = inp("bias1", [128, F], F32)
    bout = inp("bout", [128, 1], F32)
    iota_r16 = inp("iota_r16", [128, 128], F16)
    iota_c = inp("iota_c", [128, 1], F32)
    ones16 = inp("ones16", [33, 128], F16)
    ident_in = inp("ident_in", [128, 128], F16)

    out_own = nc.dram_tensor("out_own", [NVP, 1], F32, kind="ExternalOutput")

    # ---- internal DRAM
    xl0_own = nc.dram_tensor("xl0_own", [NVP, F], F16)
    xl1_own = nc.dram_tensor("xl1_own", [NVP, F], F16)
    akw = dict(addr_space="Shared") if cfg.NC > 4 else {}
    xl0_full = nc.dram_tensor("xl0_full", [cfg.NFULL, F], F16, **akw)
    xl1_full = nc.dram_tensor("xl1_full", [cfg.NFULL, F], F16, **akw)
    groups = [list(range(cfg.NC))]

    with tile.TileContext(nc) as tc:
        with (
            tc.tile_pool(name="const", bufs=1) as constp,
            tc.tile_pool(name="wpool", bufs=1) as wpool,
            tc.tile_pool(name="resid", bufs=1) as resid,
            tc.tile_pool(name="io", bufs=3) as io,
            tc.tile_pool(name="stg", bufs=2) as stg,
            tc.tile_pool(name="sel", bufs=3) as selp,
            tc.tile_pool(name="mpool", bufs=2) as mpool,
            tc.tile_pool(name="small", bufs=4) as small,
            tc.tile_pool(name="psA", bufs=2, space="PSUM") as psA,
            tc.tile_pool(name="psW", bufs=2, space="PSUM") as psW,
            tc.tile_pool(name="psS", bufs=2, space="PSUM") as psS,
        ):
            ident16 = constp.tile([128, 128], F16)
            nc.sync.dma_start(out=ident16[:], in_=ident_in[:])
            iota_row = constp.tile([128, 128], F16)
            nc.sync.dma_start(out=iota_row[:], in_=iota_r16[:])
            iota_col = constp.tile([128, 1], F32)
            nc.sync.dma_start(out=iota_col[:], in_=iota_c[:])
            ones_sb = constp.tile([33, 128], F16)
            nc.sync.dma_start(out=ones_sb[:], in_=ones16[:])
            batt = {}
            for nm, t in (("att0", att0), ("att1", att1)):
                bt = constp.tile([128, 2, 4], F16, tag=nm)
                nc.sync.dma_start(out=bt[:], in_=t[:])
                batt[nm] = bt
            bout_sb = constp.tile([128, 1], F32)
            nc.sync.dma_start(out=bout_sb[:], in_=bout[:])
            bsb = {}
            for nm, t in (("bl0", bl0), ("br0", br0), ("bias0", bias0),
                          ("bl1", bl1), ("br1", br1), ("bias1", bias1)):
                bt = constp.tile([128, F], F32, tag=nm)
                nc.sync.dma_start(out=bt[:], in_=t[:])
                bsb[nm] = bt

            def load_w(t, kdim, tag):
                n = kdim // 128
                w = wpool.tile([128, n, F], F16, tag=tag)
                nc.sync.dma_start(
                    out=w[:], in_=t.rearrange("(k p) f -> p k f", p=128))
                return w
            wl0_sb = load_w(wl0, cfg.IN_F, "wl0")
            wr0_sb = load_w(wr0, cfg.IN_F, "wr0")
            wl1_sb = load_w(wl1, F, "wl1")
            wr1_sb = load_w(wr1, F, "wr1")
            we0_sb = wpool.tile([EF, F], F16, tag="we0")
            nc.sync.dma_start(out=we0_sb[:], in_=we0[:])
            we1_sb = wpool.tile([EF, F], F16, tag="we1")
            nc.sync.dma_start(out=we1_sb[:], in_=we1[:])
            wout_sb = wpool.tile([128, 2, 1], F16, tag="wout")
            nc.sync.dma_start(out=wout_sb[:], in_=wout[:])

            # SBUF-resident xr (shared by both layers) and inter-layer h1
            xr_sb = resid.tile([128, NW, F], F16, tag="xr")
            h1_sb = resid.tile([128, NW, F], F16, tag="h1")

            # ---------------- layer-0 projections ----------------
            def phase_p1():
                for c in range(NW):
                    xk = io.tile([128, cfg.KCH, 128], F16, tag="xk")
                    nc.sync.dma_start(
                        out=xk[:],
                        in_=x_T.rearrange("(k p) n -> p k n", p=128)
                            [:, :, c * 128:(c + 1) * 128])
                    plr = psA.tile([128, 2, F], F32, tag="pm", bufs=2)
                    for k in range(cfg.KCH):
                        nc.tensor.matmul(plr[:, 0, :], lhsT=xk[:, k, :],
                                         rhs=wl0_sb[:, k, :],
                                         start=(k == 0), stop=(k == cfg.KCH - 1),
                                         skip_group_check=True)
                    for k in range(cfg.KCH):
                        nc.tensor.matmul(plr[:, 1, :], lhsT=xk[:, k, :],
                                         rhs=wr0_sb[:, k, :],
                                         start=(k == 0), stop=(k == cfg.KCH - 1),
                                         skip_group_check=True)
                    ol = io.tile([128, F], F16, tag="oxl")
                    nc.vector.tensor_add(out=ol[:], in0=plr[:, 0, :],
                                         in1=bsb["bl0"][:])
                    nc.vector.tensor_add(out=xr_sb[:, c, :], in0=plr[:, 1, :],
                                         in1=bsb["br0"][:])
                    nc.sync.dma_start(out=xl0_own[c * 128:(c + 1) * 128, :],
                                      in_=ol[:])

            # ---------------- edge pass (shared for both layers) ------------
            def edge_pass(layer, xl_full, we_sb, att_sb, bias_sb, cc=None):
                NCH = cfg.NCH
                HC = SWC // NCH
                for w in range(NW):
                    idx_sb = io.tile([128, NCH, cfg.HSW // 16], I16,
                                     tag="idx")
                    nc.sync.dma_start(out=idx_sb[:], in_=idx2[w])
                    ea_sb = io.tile([EF + 1, SW], F16, tag="ea")
                    nc.sync.dma_start(out=ea_sb[:], in_=eat[w])
                    dstf_sb = io.tile([128, SWC], F16, tag="dstf")
                    nc.sync.dma_start(out=dstf_sb[:], in_=dstf[w])
                    stage = stg.tile([128, SWC, F], F16, tag="stage")
                    for q in range(NCH):
                        # chunks 0..NCH/2 gather low table rows; rest high.
                        tbl = (xl_full[:cfg.HALF, :] if q < NCH // 2
                               else xl_full[cfg.HALF:, :])
                        g1 = nc.gpsimd.dma_gather(
                            out_ap=stage[:, q * HC:(q + 1) * HC, :],
                            in_ap=tbl,
                            idxs_ap=idx_sb[:, q, :], num_idxs=cfg.HSW,
                            num_idxs_reg=cfg.HSW, elem_size=F)
                        if cc is not None:
                            bass._add_dep_helper(
                                g1.ins, cc.ins, sync=True,
                                reason="gather reads AllGather output")
                    # pall: [0:256] weighted sum | [256:260] denominator
                    pall = psW.tile([128, 260], F32, tag="pall")
                    for j in range(J):
                        ed = ea_sb[:, j * T_E:(j + 1) * T_E]
                        drow = psS.tile([128, T_E], F32, tag="tmp")
                        nc.tensor.matmul(drow[:], lhsT=ones_sb[EF:EF + 1, :],
                                         rhs=ed[EF:EF + 1, :],
                                         start=True, stop=True,
                                         skip_group_check=True)
                        st_j = selp.tile([128, T_E], F16, tag="st")
                        nc.vector.tensor_tensor(
                            out=st_j[:],
                            in0=iota_col[:].to_broadcast([128, T_E]),
                            in1=drow[:], op=mybir.AluOpType.is_equal)
                        s_j = selp.tile([128, G, 128], F16, tag="s")
                        nc.vector.tensor_tensor(
                            out=s_j[:],
                            in0=dstf_sb[:, G * j:G * j + G]
                                .unsqueeze(-1).to_broadcast([128, G, 128]),
                            in1=iota_row[:].unsqueeze(1)
                                .to_broadcast([128, G, 128]),
                            op=mybir.AluOpType.is_equal)
                        m_t = mpool.tile([128, 2, T_E], F16, tag="m")
                        for h in range(2):
                            pm = psA.tile([128, T_E], F32, tag="pm", bufs=2)
                            nc.tensor.matmul(
                                pm[:], lhsT=we_sb[:, h * 128:(h + 1) * 128],
                                rhs=ed[:EF, :], start=True, stop=False,
                                skip_group_check=True)
                            nc.tensor.matmul(
                                pm[:],
                                lhsT=xr_sb[:, w, h * 128:(h + 1) * 128],
                                rhs=st_j[:], start=False, stop=False,
                                skip_group_check=True)
                            # transpose-accumulate gathered xl[src] via
                            # identity matmul (stage^T @ I), f32 accumulate
                            for g in range(G):
                                nc.tensor.matmul(
                                    pm[:, g * 128:(g + 1) * 128],
                                    lhsT=stage[:, G * j + g,
                                               h * 128:(h + 1) * 128],
                                    rhs=ident16[:],
                                    start=False, stop=(g == G - 1),
                                    skip_group_check=True)
                            rp = mpool.tile([128, T_E], F16, tag="rp")
                            nc.scalar.activation(
                                rp[:], pm[:], mybir.ActivationFunctionType.Relu,
                                scale=1.0 - NEG_SLOPE)
                            nc.vector.scalar_tensor_tensor(
                                out=m_t[:, h, :], in0=pm[:], scalar=NEG_SLOPE,
                                in1=rp[:], op0=mybir.AluOpType.mult,
                                op1=mybir.AluOpType.add)
                        plog = psS.tile([128, 16], F32, tag="tmp")
                        for g in range(G):
                            for h in range(2):
                                nc.tensor.matmul(
                                    plog[:, 4 * g:4 * g + 4],
                                    lhsT=m_t[:, h, g * 128:(g + 1) * 128],
                                    rhs=att_sb[:, h, :],
                                    start=(h == 0), stop=(h == 1),
                                    skip_group_check=True)
                        v = small.tile([128, G, 260], F16, tag="v")
                        nc.scalar.activation(
                            v[:, :, 256:260],
                            plog[:].rearrange("p (g h) -> p g h", g=4),
                            mybir.ActivationFunctionType.Exp)
                        nc.vector.tensor_tensor(
                            out=v[:, :, :256].rearrange(
                                "p g (h c) -> p g h c", h=4),
                            in0=stage[:, G * j:G * (j + 1), :].rearrange(
                                "p g (h c) -> p g h c", h=4),
                            in1=v[:, :, 256:260]
                                .unsqueeze(-1).to_broadcast([128, G, 4, 64]),
                            op=mybir.AluOpType.mult)
                        for g in range(G):
                            nc.tensor.matmul(
                                pall[:, 0:260], lhsT=s_j[:, g, :],
                                rhs=v[:, g, :],
                                start=(j == 0 and g == 0),
                                stop=(j == J - 1 and g == G - 1),
                                skip_group_check=True)
                    rdf = small.tile([128, 4], F32, tag="rdf")
                    nc.vector.tensor_scalar_add(out=rdf[:],
                                                in0=pall[:, 256:260],
                                                scalar1=EPS)
                    rden = small.tile([128, 4], F32, tag="rden")
                    nc.vector.reciprocal(out=rden[:], in_=rdf[:])
                    hs = small.tile([128, F], F32, tag="hs")
                    nc.vector.tensor_tensor(
                        out=hs.rearrange("p (h c) -> p h c", h=4),
                        in0=pall[:, 0:256].rearrange("p (h c) -> p h c", h=4),
                        in1=rden.unsqueeze(-1).to_broadcast([128, 4, 64]),
                        op=mybir.AluOpType.mult)
                    hb = small.tile([128, F], F32, tag="hb")
                    nc.vector.tensor_add(out=hb[:], in0=hs[:], in1=bias_sb[:])
                    if layer == 0:
                        nc.scalar.activation(h1_sb[:, w, :], hb[:],
                                             mybir.ActivationFunctionType.Relu)
                    else:
                        h_out = small.tile([128, F], F16, tag="hout")
                        nc.scalar.activation(h_out[:], hb[:],
                                             mybir.ActivationFunctionType.Relu)
                        po = psS.tile([128, 1], F32, tag="tmp")
                        for h in range(2):
                            pt = psS.tile([128, 128], F16, tag="tmp")
                            nc.tensor.matmul(pt[:],
                                             lhsT=h_out[:, h * 128:(h + 1) * 128],
                                             rhs=ident16[:], is_transpose=True,
                                             start=True, stop=True,
                                             skip_group_check=True)
                            h2T = small.tile([128, 128], F16, tag="h2T")
                            nc.vector.tensor_copy(out=h2T[:], in_=pt[:])
                            nc.tensor.matmul(po[:], lhsT=h2T[:],
                                             rhs=wout_sb[:, h, :],
                                             start=(h == 0), stop=(h == 1),
                                             skip_group_check=True)
                        o_sb = small.tile([128, 1], F32, tag="osb")
                        nc.vector.tensor_scalar(
                            out=o_sb[:], in0=po[:], scalar1=bout_sb[:, :1],
                            scalar2=None, op0=mybir.AluOpType.add)
                        nc.sync.dma_start(out=out_own[w * W:(w + 1) * W, :],
                                          in_=o_sb[:])

            # ---------------- layer-1 projections ----------------
            def phase_p4():
                for c in range(NW):
                    h1T = io.tile([128, 2, 128], F16, tag="h1T")
                    for h in range(2):
                        pt = psS.tile([128, 128], F16, tag="tmp")
                        nc.tensor.matmul(pt[:],
                                         lhsT=h1_sb[:, c, h * 128:(h + 1) * 128],
                                         rhs=ident16[:], is_transpose=True,
                                         start=True, stop=True,
                                         skip_group_check=True)
                        nc.vector.tensor_copy(out=h1T[:, h, :], in_=pt[:])
                    plr = psA.tile([128, 2, F], F32, tag="pm", bufs=2)
                    for h in range(2):
                        nc.tensor.matmul(plr[:, 0, :], lhsT=h1T[:, h, :],
                                         rhs=wl1_sb[:, h, :],
                                         start=(h == 0), stop=(h == 1),
                                         skip_group_check=True)
                    for h in range(2):
                        nc.tensor.matmul(plr[:, 1, :], lhsT=h1T[:, h, :],
                                         rhs=wr1_sb[:, h, :],
                                         start=(h == 0), stop=(h == 1),
                                         skip_group_check=True)
                    ol = io.tile([128, F], F16, tag="oxl")
                    nc.vector.tensor_add(out=ol[:], in0=plr[:, 0, :],
                                         in1=bsb["bl1"][:])
                    nc.vector.tensor_add(out=xr_sb[:, c, :], in0=plr[:, 1, :],
                                         in1=bsb["br1"][:])
                    nc.sync.dma_start(out=xl1_own[c * 128:(c + 1) * 128, :],
                                      in_=ol[:])

            def phase_ag(xl_own, xl_full):
                # Barrier ensures the xl shard DMA writes landed before the
                # collective reads them. No post-AG barrier: the only
                # consumers of xl_full are the dma_gathers, which follow the
                # collective in GPSIMD program order — so window preambles
                # (loads, selector builds) overlap the collective.
                tc.strict_bb_all_engine_barrier()
                cc = nc.gpsimd.collective_compute(
                    "AllGather", mybir.AluOpType.bypass, replica_groups=groups,
                    ins=[xl_own[:]], outs=[xl_full[:]])
                tc.strict_bb_all_engine_barrier()
                return cc

            for _rep in range(cfg.reps):
                cc0 = cc1 = None
                if "p1" in cfg.phases:
                    phase_p1()
                if "ag0" in cfg.phases:
                    cc0 = phase_ag(xl0_own, xl0_full)
                if "e0" in cfg.phases:
                    edge_pass(0, xl0_full, we0_sb, batt["att0"], bsb["bias0"],
                              cc=cc0)
                tc.strict_bb_all_engine_barrier()
                if "p4" in cfg.phases:
                    phase_p4()
                if "ag1" in cfg.phases:
                    cc1 = phase_ag(xl1_own, xl1_full)
                if "e1" in cfg.phases:
                    edge_pass(1, xl1_full, we1_sb, batt["att1"], bsb["bias1"],
                              cc=cc1)
                # Rep-end barrier: guarantees the collectives' reads and all
                # in-flight gather DMAs completed before the next rep (or
                # program end) rewrites their sources.
                tc.strict_bb_all_engine_barrier()
    return P


_CACHE = {}


def _get_compiled(cfg):
    key = (cfg.N, cfg.E, cfg.IN_F, cfg.NC, cfg.J, cfg.reps, cfg.phases,
           cfg.NWX)
    if key not in _CACHE:
        import os as _os
        nc = bacc.Bacc("TRN2", target_bir_lowering=False, debug=False,
                       num_devices=cfg.NC,
                       dynamic_dma_scratch_size=int(
                           _os.environ.get("K_SCRATCH", "49152")),
                       num_swdge_queues=int(_os.environ.get("K_QUEUES", "1")))
        build_program(cfg, nc)
        nc.compile()
        _CACHE[key] = nc
    return _CACHE[key]


def make_in_maps(cfg, inputs, cores_pre):
    """Per-core input dicts."""
    x = np.asarray(inputs["x"], np.float32)
    H, C, F = cfg.H, cfg.C, cfg.F
    att_blk = {}
    for li in (0, 1):
        att = np.asarray(inputs[f"att{li}"], np.float32)   # [H, C]
        A = np.zeros((2 * 128, 4), np.float32)
        for h in range(H):
            A[h * C:(h + 1) * C, h] = att[h]
        att_blk[li] = np.ascontiguousarray(
            A.reshape(2, 128, 4).transpose(1, 0, 2)).astype(np.float16)
    iota_r16 = np.tile(np.arange(128, dtype=np.float16)[None, :], (128, 1))
    iota_c = np.arange(128, dtype=np.float32).reshape(128, 1)
    ones16 = np.ones((33, 128), np.float16)
    rep = lambda v: np.tile(np.asarray(v, np.float32)[None, :], (128, 1))
    f16 = lambda v: np.asarray(v, np.float32).astype(np.float16)
    common = dict(
        wl0=f16(inputs["W_l0"]), wr0=f16(inputs["W_r0"]),
        we0=f16(inputs["W_e0"]), wl1=f16(inputs["W_l1"]),
        wr1=f16(inputs["W_r1"]), we1=f16(inputs["W_e1"]),
        wout=f16(inputs["W_out"]).reshape(2, 128, 1).transpose(1, 0, 2).copy(),
        att0=att_blk[0], att1=att_blk[1],
        bl0=rep(inputs["b_l0"]), br0=rep(inputs["b_r0"]),
        bias0=rep(inputs["bias0"]), bl1=rep(inputs["b_l1"]),
        br1=rep(inputs["b_r1"]), bias1=rep(inputs["bias1"]),
        bout=np.tile(np.asarray(inputs["b_out"], np.float32).reshape(1, 1),
                     (128, 1)),
        iota_r16=iota_r16, iota_c=iota_c, ones16=ones16,
        ident_in=np.eye(128, dtype=np.float16),
    )
    in_maps = []
    for c in range(cfg.NC):
        pre = cores_pre[c]
        xs = np.zeros((cfg.NVP, cfg.IN_F), np.float32)
        xs[pre["rows"]] = x[c * cfg.NV:(c + 1) * cfg.NV]
        m = dict(common)
        m.update(x_T=np.ascontiguousarray(xs.T).astype(np.float16),
                 idx2=pre["idx2"], dstf=pre["dstf"],
                 eat=pre["eat"])
        in_maps.append(m)
    return in_maps


def _run(cfg, inputs):
    cores_pre = preprocess(cfg, inputs["edge_index"], inputs["edge_attr"])
    in_maps = make_in_maps(cfg, inputs, cores_pre)
    nc = _get_compiled(cfg)
    res = run_bass_kernel_spmd(nc, in_maps, core_ids=list(range(cfg.NC)))
    outs = []
    for c in range(cfg.NC):
        outs.append(res.results[c]["out_own"][cores_pre[c]["rows"]])
    return np.concatenate(outs, 0).astype(np.float32)


def pick_cfg(edge_index, reps=1):
    """Smallest (J, NWX) whose 2D-balanced packing fits this graph."""
    src = np.asarray(edge_index[0], np.int64)
    dst = np.asarray(edge_index[1], np.int64)
    for J, NWX in ((4, 1), (5, 1), (5, 2), (6, 2), (8, 4)):
        cfg = Cfg(N=50000, E=800000, IN_F=512, NC=8, J=J, reps=reps, NWX=NWX)
        is_lo = src < (cfg.N // 2)
        deg_lo = np.bincount(dst[is_lo], minlength=cfg.N)
        deg_hi = np.bincount(dst[~is_lo], minlength=cfg.N)
        ok = True
        for c in range(cfg.NC):
            sl = slice(c * cfg.NV, (c + 1) * cfg.NV)
            try:
                _balance_windows(cfg, deg_lo[sl], deg_hi[sl])
            except AssertionError:
                ok = False
                break
        if ok:
            return cfg
    return cfg


def kernel(**inputs):
    cfg = pick_cfg(inputs["edge_index"])
    return _run(cfg, inputs)

